# revision 1
# baseline (speedup 1.0000x reference)
"""EventVolumeSurface trilinear voxel-grid kernel for Trainium2 (Bass/Tile).

Strategy (data-parallel over batch, 1 batch -> 1 NeuronCore):
  - Host: shard events by batch id, compute bucket keys (time-segment s in
    [0,9), y-tile q in [0,4), x-tile r in [0,5)), duplicate events that
    straddle a y/x tile boundary (the trilinear hat auto-masks out-of-tile
    taps, so duplication is exact), sort into (s,q,r) buckets, pad each
    bucket to a multiple of 128 slots, and lay out slot-major [128, T]
    arrays of y, x, t, polarity.
  - Device: per event column, compute t' = a*t + b (t* in [0,9]), per
    segment frac = t' - s, kt1 = frac*pol, kt0 = pol - kt1.  Per tile of
    128 events: |IOTA_q - y| (GPSIMD), |IOTA_r - x| (DVE), hat = relu(1-d)
    (ACT, batched over groups of tiles), rhs = [kt0*hatX | kt1*hatX] (DVE),
    then one PE matmul psum[y,256] += hatY^T @ rhs accumulating the two
    adjacent bin planes of the segment.  PSUM is drained per (s,q) into an
    SBUF-resident [10,480,640] grid which is DMA'd to DRAM at the end.

The kernel program is compiled per bucket-schedule (shared across all 8
cores: per-bucket tile counts are the max over cores).
"""

import os
import sys

import numpy as np

sys.path.insert(0, "/opt/trn_rl_repo")

import concourse.bass as bass
import concourse.bacc as bacc
import concourse.mybir as mybir
import concourse.tile as tile
from concourse.bass_utils import run_bass_kernel_spmd

H, W, BINS = 480, 640, 10
NSEG = BINS - 1          # 9 time segments (events with t*=9 fold into seg 8)
P = 128
NQ = (H + P - 1) // P    # 4 y-tiles
NR = (W + P - 1) // P    # 5 x-tiles
NKEY = NSEG * NQ * NR    # 180 buckets
N_CORES = 8
GROUP = 8               # tiles per batched abs/relu/clamp op

F32 = mybir.dt.float32
F16 = mybir.dt.float16
MM_DT = F16              # PE operand dtype: fp16 is full-rate at any N
DY_GPS = bool(int(os.environ.get("EVS_DY_GPS", "1")))
TINY = bool(int(os.environ.get("EVS_TINY", "0")))  # timing diagnostic only

_prog_cache: dict = {}


def _host_prep(ev):
    """Bucket one batch's events; returns (counts[NKEY], packing arrays)."""
    if ev.shape[0] == 0:
        # degenerate batch: dummy zero-polarity events (contribute 0)
        ev = np.array([[0.0, 0.0, 0.25, 0.0, 0.0],
                       [0.0, 0.0, 0.75, 0.0, 0.0]], np.float32)
    x = ev[:, 0].astype(np.float32)
    y = ev[:, 1].astype(np.float32)
    t = ev[:, 2].astype(np.float32)
    p = ev[:, 3].astype(np.float32)
    t0 = t[0]
    tN = t[-1]
    denom = np.float32(tN - t0)
    if denom > 0:
        a = np.float32(np.float32(BINS - 1) / denom)
    else:
        a = np.float32(0.0)
    b = np.float32(-t0 * a)
    tp = (t * a + b).astype(np.float32)
    s = np.clip(np.floor(tp).astype(np.int32), 0, NSEG - 1)

    iy = np.floor(y).astype(np.int32)
    icy = np.ceil(y).astype(np.int32)
    ix = np.floor(x).astype(np.int32)
    icx = np.ceil(x).astype(np.int32)
    qf, qc = iy >> 7, icy >> 7
    rf, rc = ix >> 7, icx >> 7
    n = len(x)
    idx0 = np.arange(n, dtype=np.int64)

    ys = qf != qc
    xs = rf != rc
    both = ys & xs
    inst_idx = np.concatenate([idx0, idx0[ys], idx0[xs], idx0[both]])
    inst_q = np.concatenate([qf, qc[ys], qf[xs], qc[both]])
    inst_r = np.concatenate([rf, rf[ys], rc[xs], rc[both]])
    key = (s[inst_idx] * NQ + inst_q) * NR + inst_r
    counts = np.bincount(key, minlength=NKEY)
    return counts, (x, y, t, p, a, b, inst_idx, key)


def _pack_core(pack, tiles_per_key, T_tot):
    x, y, t, p, a, b, inst_idx, key = pack
    col0 = np.zeros(NKEY + 1, np.int64)
    col0[1:] = np.cumsum(tiles_per_key)
    order = np.argsort(key, kind="stable")
    skey = key[order]
    sidx = inst_idx[order]
    # rank within each key group
    group_start = np.searchsorted(skey, np.arange(NKEY))
    rank = np.arange(len(skey)) - group_start[skey]
    slot = col0[skey] * P + rank
    part = (slot % P).astype(np.int64)
    col = (slot // P).astype(np.int64)

    # two packed inputs: persistent y|x and prologue-only t|p|(a,b)
    YX = np.zeros((P, 2 * T_tot), np.float32)
    YX[part, col] = y[sidx]
    YX[part, T_tot + col] = x[sidx]
    TP = np.zeros((P, 2 * T_tot + 2), np.float32)
    TP[part, col] = t[sidx]
    TP[part, T_tot + col] = p[sidx]
    TP[:, 2 * T_tot] = a
    TP[:, 2 * T_tot + 1] = b
    return {"ev_yx": YX, "ev_tp": TP}


def _build_program(tiles_per_key, T_tot):
    nc = bacc.Bacc("TRN2", debug=False)
    yx_d = nc.dram_tensor("ev_yx", [P, 2 * T_tot], F32, kind="ExternalInput")
    tp_d = nc.dram_tensor("ev_tp", [P, 2 * T_tot + 2], F32,
                          kind="ExternalInput")
    out_d = nc.dram_tensor("out", [BINS, H, W], F32, kind="ExternalOutput")

    col0 = np.zeros(NKEY + 1, np.int64)
    col0[1:] = np.cumsum(tiles_per_key)
    # per-segment column ranges (keys are s-major)
    seg_c0 = [int(col0[s * NQ * NR]) for s in range(NSEG)]
    seg_c1 = [int(col0[(s + 1) * NQ * NR]) for s in range(NSEG)]

    Alu = mybir.AluOpType
    Act = mybir.ActivationFunctionType

    with tile.TileContext(nc) as tc:
        with (
            tc.tile_pool(name="persist", bufs=1) as persist,
            tc.tile_pool(name="grid", bufs=1) as gridp,
            tc.tile_pool(name="psum", bufs=2, space="PSUM") as psump,
        ):
            # --- load inputs (ev_tp only lives through the prologue)
            yxt = persist.tile([P, 2 * T_tot], F32, tag="yxt")
            yt = yxt[:, 0:T_tot]
            xt = yxt[:, T_tot:2 * T_tot]
            nc.sync.dma_start(out=yxt[:], in_=yx_d[:])

            # --- constants: per-tile iota tables 128q + c and 128r + c
            ioq = []
            ior = []
            for q in range(NQ):
                ti = persist.tile([P, P], mybir.dt.int32, tag=f"ioqi{q}")
                nc.gpsimd.iota(ti[:], pattern=[[1, P]], base=q * P,
                               channel_multiplier=0)
                tf = persist.tile([P, P], F32, tag=f"ioqf{q}")
                nc.vector.tensor_copy(tf[:], ti[:])
                ioq.append(tf)
            for r in range(NR):
                ti = persist.tile([P, P], mybir.dt.int32, tag=f"iori{r}")
                nc.gpsimd.iota(ti[:], pattern=[[1, P]], base=r * P,
                               channel_multiplier=0)
                tf = persist.tile([P, P], F32, tag=f"iorf{r}")
                nc.vector.tensor_copy(tf[:], ti[:])
                ior.append(tf)

            # --- preprocess: t' = a*t + b ; frac = t' - s ;
            #     nk1 = -frac*pol ; nk0 = -(pol - frac*pol)
            #     (negated because the muls read -hat_x: (-hat_x)*(-kt)=hat_x*kt)
            nk0 = persist.tile([P, T_tot], F32, tag="nk0")
            nk1 = persist.tile([P, T_tot], F32, tag="nk1")
            with tc.tile_pool(name="prolog", bufs=1) as prolog:
                tpt = prolog.tile([P, 2 * T_tot + 2], F32, tag="tpt")
                tt = tpt[:, 0:T_tot]
                pt = tpt[:, T_tot:2 * T_tot]
                ab = tpt[:, 2 * T_tot:2 * T_tot + 2]
                nc.sync.dma_start(out=tpt[:], in_=tp_d[:])
                tc.strict_bb_all_engine_barrier()
                nc.vector.tensor_scalar(nk1[:], tt, ab[:, 0:1], ab[:, 1:2],
                                        op0=Alu.mult, op1=Alu.add)
                for s in range(NSEG):
                    c0, c1 = seg_c0[s], seg_c1[s]
                    if c1 > c0:
                        nc.vector.tensor_scalar(nk1[:, c0:c1], nk1[:, c0:c1],
                                                float(s), None,
                                                op0=Alu.subtract)
                # nk1 holds frac; kt1 = frac*pol; nk1 := -kt1
                nc.vector.tensor_tensor(nk1[:], nk1[:], pt, op=Alu.mult)
                nc.vector.tensor_scalar(nk1[:], nk1[:], -1.0, None,
                                        op0=Alu.mult)
                # nk0 = -(pol - kt1) = -pol - nk1
                nc.vector.tensor_tensor(nk0[:], nk1[:], pt, op=Alu.add)
                nc.vector.tensor_scalar(nk0[:], nk0[:], -1.0, None,
                                        op0=Alu.mult)

            tc.strict_bb_all_engine_barrier()

            # --- the SBUF-resident output grid [128, BINS*NQ*640]
            V = gridp.tile([P, BINS * NQ * W], F32, tag="V")

            # --- main loops (EVS_REPEAT > 1 is a timing-only mode: output
            #     values are wrong for the `add` drains but timing per pass
            #     is identical)
            repeat = int(os.environ.get("EVS_REPEAT", "1"))
            with (
                tc.tile_pool(name="ay", bufs=4) as ayp,
                tc.tile_pool(name="ax", bufs=4) as axp,
                tc.tile_pool(name="hy", bufs=4) as hyp,
                tc.tile_pool(name="hx", bufs=4) as hxp,
                tc.tile_pool(name="rhs", bufs=8) as rhsp,
            ):
             for _rep in range(repeat):
              for s in range(NSEG):
                for q in range(NQ):
                    psum_t = psump.tile([P, NR * 256], F32, tag="ps")
                    for r in range(NR):
                        k = (s * NQ + q) * NR + r
                        ntile = int(tiles_per_key[k])
                        cbase = int(col0[k])
                        for g0 in range(0, ntile, GROUP):
                            gn = min(GROUP, ntile - g0)
                            gw = gn * P
                            ayg = ayp.tile([P, GROUP * P], F32, tag="ayg")
                            axg = axp.tile([P, GROUP * P], F32, tag="axg")
                            TW = 8 if TINY else P
                            for j in range(gn):
                                c = cbase + g0 + j
                                if DY_GPS:
                                    nc.gpsimd.tensor_tensor(
                                        ayg[:, j * P:j * P + TW], ioq[q][:, :TW],
                                        yt[:, c:c + 1].to_broadcast([P, TW]),
                                        op=Alu.subtract)
                                else:
                                    nc.vector.tensor_scalar(
                                        ayg[:, j * P:j * P + TW], ioq[q][:, :TW],
                                        yt[:, c:c + 1], None, op0=Alu.subtract)
                                nc.vector.tensor_scalar(
                                    axg[:, j * P:j * P + TW], ior[r][:, :TW],
                                    xt[:, c:c + 1], None, op0=Alu.subtract)
                            hyg = hyp.tile([P, GROUP * P], MM_DT, tag="hyg")
                            nhxg = hxp.tile([P, GROUP * P], MM_DT, tag="nhxg")
                            # |d| in place (ACT); hat_y = relu(1-|dy|) (ACT);
                            # -hat_x = min(|dx|-1, 0)  (DVE, batched)
                            bw = gn * P if not TINY else gn * 8
                            nc.scalar.activation(ayg[:, :bw], ayg[:, :bw],
                                                 Act.Abs)
                            nc.scalar.activation(axg[:, :bw], axg[:, :bw],
                                                 Act.Abs)
                            nc.scalar.activation(hyg[:, :bw], ayg[:, :bw],
                                                 Act.Relu, bias=1.0, scale=-1.0)
                            nc.vector.tensor_scalar(nhxg[:, :bw], axg[:, :bw],
                                                    1.0, 0.0, op0=Alu.subtract,
                                                    op1=Alu.min)
                            for j in range(gn):
                                c = cbase + g0 + j
                                rhs = rhsp.tile([P, 256], MM_DT, tag="rhs")
                                nc.vector.tensor_scalar(
                                    rhs[:, 0:TW], nhxg[:, j * P:j * P + TW],
                                    nk0[:, c:c + 1], None, op0=Alu.mult)
                                nc.vector.tensor_scalar(
                                    rhs[:, P:P + TW], nhxg[:, j * P:j * P + TW],
                                    nk1[:, c:c + 1], None, op0=Alu.mult)
                                first = (g0 + j == 0)
                                last = (g0 + j == ntile - 1)
                                nc.tensor.matmul(
                                    psum_t[:, r * 256:(r + 1) * 256],
                                    lhsT=hyg[:, j * P:(j + 1) * P],
                                    rhs=rhs[:],
                                    start=first, stop=last)
                    # drain psum -> V for plane s (half 0) and s+1 (half 1)
                    pv = psum_t[:].rearrange("p (r h c) -> p h r c", r=NR, h=2,
                                             c=P)
                    for half, plane in ((0, s), (1, s + 1)):
                        base = (plane * NQ + q) * W
                        vv = V[:, base:base + W].rearrange("p (r c) -> p r c",
                                                           c=P)
                        if (half == 0 and s == 0) or half == 1:
                            nc.scalar.copy(vv, pv[:, half])
                        else:
                            nc.vector.tensor_tensor(vv, vv, pv[:, half],
                                                    op=Alu.add)
                # plane s is final after its half-0 drains: stream it out now
                # so the 12.3MB writeback overlaps remaining compute
                if _rep == repeat - 1:
                    planes = [s] if s < NSEG - 1 else [s, s + 1]
                    for bin_i in planes:
                        for q in range(NQ):
                            rows = min(P, H - q * P)
                            base = (bin_i * NQ + q) * W
                            nc.sync.dma_start(
                                out=out_d[bin_i, q * P:q * P + rows, :],
                                in_=V[0:rows, base:base + W])
    nc.finalize()
    return nc


def kernel(events, lengths):
    events = np.ascontiguousarray(events, dtype=np.float32)
    lengths = np.asarray(lengths)
    B = int(lengths.shape[0])
    offs = np.zeros(B + 1, np.int64)
    offs[1:] = np.cumsum(lengths)

    packs = []
    counts = np.zeros((B, NKEY), np.int64)
    for bi in range(B):
        c, pk = _host_prep(events[offs[bi]:offs[bi + 1]])
        counts[bi] = c
        packs.append(pk)

    tiles_per_key = np.maximum(1, -(-counts.max(axis=0) // P)).astype(np.int64)
    T_tot = int(tiles_per_key.sum())

    key = (tuple(tiles_per_key.tolist()), T_tot,
           os.environ.get("EVS_REPEAT", "1"), TINY)
    if key not in _prog_cache:
        _prog_cache[key] = _build_program(tiles_per_key, T_tot)
    nc = _prog_cache[key]

    in_maps = [_pack_core(pk, tiles_per_key, T_tot) for pk in packs]
    trace = bool(int(os.environ.get("EVS_TRACE", "0")))
    res = run_bass_kernel_spmd(nc, in_maps, core_ids=list(range(B)),
                               trace=trace)
    global last_results
    last_results = res
    out = np.stack([r["out"] for r in res.results], axis=0)
    return out.astype(np.float32)


last_results = None


if __name__ == "__main__":
    # tiny smoke test with synthetic events
    rng = np.random.default_rng(0)
    B0, NP0 = 8, 2000
    N0 = B0 * NP0
    x = rng.uniform(0, W - 1, N0).astype(np.float32)
    y = rng.uniform(0, H - 1, N0).astype(np.float32)
    t = np.sort(rng.uniform(0, 1, (B0, NP0)).astype(np.float32), axis=1).ravel()
    p = (2.0 * rng.integers(0, 2, N0) - 1).astype(np.float32)
    b = np.repeat(np.arange(B0), NP0).astype(np.float32)
    ev = np.stack([x, y, t, p, b], axis=1)
    ln = np.full(B0, NP0, np.int32)
    out = kernel(ev, ln)
    # numpy reference
    ref = np.zeros((B0, BINS, H, W), np.float64)
    for bi in range(B0):
        sl = slice(bi * NP0, (bi + 1) * NP0)
        xx, yy, tt2, pp = x[sl], y[sl], t[sl], p[sl]
        t0, tN = tt2[0], tt2[-1]
        ts = (BINS - 1) * np.clip((tt2 - t0) / (tN - t0), 0, 1)
        import itertools
        for xr_f, yr_f, br_f in itertools.product([np.floor, np.ceil], repeat=3):
            xr, yr, br = xr_f(xx), yr_f(yy), br_f(ts)
            valid = (((xr != xx) | (xr_f is np.floor))
                     & ((yr != yy) | (yr_f is np.floor))
                     & ((br != ts) | (br_f is np.floor))
                     & (xr < W) & (yr < H) & (br < BINS))
            kb = lambda a_: np.maximum(0, 1 - np.abs(a_))
            val = np.where(valid, pp * kb(xr - xx) * kb(yr - yy) * kb(br - ts), 0)
            np.add.at(ref[bi].ravel(),
                      np.where(valid, (xr + yr * W + br * H * W).astype(np.int64), 0),
                      val)
    err = np.abs(out - ref).max() / max(1e-9, np.abs(ref).max())
    print("smoke rel err:", err)



# revision 4
# speedup vs baseline: 1.4729x; 1.4729x over previous
"""EventVolumeSurface trilinear voxel-grid kernel for Trainium2 (Bass/Tile).

Strategy (data-parallel over batch, 1 batch -> 1 NeuronCore):
  Host: shard events by batch id, bucket by (time-segment s in [0,9),
  y-stripe q in [0,8) of 64 rows; events straddling a y-stripe boundary are
  duplicated - the trilinear hat masks the out-of-stripe tap), then SORT each
  bucket's events by x.  Cut into 128-event tiles; each tile gets a
  compile-time x-window [base_t, base_t+w_t) covering all 8 cores' taps
  (w_t ~ 16-40 cols).  Pack slot-major [128, T] arrays of y, x-base-relative,
  t, polarity.

  Device, per tile of 128 events (ops batched over groups of <=8 tiles):
    dy   = iota_y - y            (Pool,  [128, G*64])
    ady  = |dy|                  (ACT Abs)
    nhy  = min(ady,1) - 1        (DVE fused tensor_scalar, fp16) = -hat_y
    dx   = iota_w - xloc         (Pool/DVE)
    adx  = |dx|                  (ACT Abs)
    mxc  = min(adx,1)            (DVE, fp16)
    r0   = g0*mxc - g0           (DVE fused 2-scalar TS, fp16) = -g0*hat_x
    r1   = g1*mxc - g1           where g0 = p*(1-frac), g1 = p*frac
    psum[64, 1280] += nhy^T @ [r0 | r1]  (PE, N=w_t per bin half)
  so psum accumulates +g*hat_y*hat_x for the two adjacent bin planes.  PSUM
  is drained per (s,q) into an SBUF-resident [10,480,640] grid (pre-zeroed)
  which is DMA'd out plane-by-plane as planes finalize.
"""

import os
import sys

import numpy as np

sys.path.insert(0, "/opt/trn_rl_repo")

import concourse.bass as bass
import concourse.bacc as bacc
import concourse.mybir as mybir
import concourse.tile as tile
from concourse.bass_utils import run_bass_kernel_spmd

H, W, BINS = 480, 640, 10
NSEG = BINS - 1          # 9 time segments (events with t*=9 fold into seg 8)
P = 128
QS = 64                  # y-stripe height
NQ8 = 8                  # ceil(480/64) = 7.5 -> 8 stripes (last half-used)
NKEY = NSEG * NQ8        # 72 buckets
NQ4 = 4                  # V-grid column blocks of 128 rows
N_CORES = 8
GROUP = 8                # tiles per batched op
XCAP = 1280              # max f32 columns per batched x-op slab (SBUF budget)

F32 = mybir.dt.float32
F16 = mybir.dt.float16

# fraction of x-diff (dx) batched ops issued on Pool engine (rest on DVE)
DX_POOL_NUM = int(os.environ.get("EVS_DX_POOL_NUM", "9"))
DX_POOL_DEN = 10

_prog_cache: dict = {}


def _host_prep(ev):
    """Bucket one batch's events by (s, q64); returns counts and raw data."""
    if ev.shape[0] == 0:
        ev = np.array([[0.0, 0.0, 0.25, 0.0, 0.0],
                       [0.0, 0.0, 0.75, 0.0, 0.0]], np.float32)
    x = ev[:, 0].astype(np.float32)
    y = ev[:, 1].astype(np.float32)
    t = ev[:, 2].astype(np.float32)
    p = ev[:, 3].astype(np.float32)
    t0 = t[0]
    tN = t[-1]
    denom = np.float32(tN - t0)
    if denom > 0:
        a = np.float32(np.float32(NSEG) / denom)
    else:
        a = np.float32(0.0)
    b = np.float32(-t0 * a)
    tp = (t * a + b).astype(np.float32)
    s = np.clip(np.floor(tp).astype(np.int32), 0, NSEG - 1)

    iy = np.floor(y).astype(np.int32)
    qf = iy >> 6
    qc = (iy + 1) >> 6
    n = len(x)
    idx0 = np.arange(n, dtype=np.int64)
    ys = qf != qc
    inst_idx = np.concatenate([idx0, idx0[ys]])
    inst_q = np.concatenate([qf, qc[ys]])
    key = s[inst_idx] * NQ8 + inst_q
    counts = np.bincount(key, minlength=NKEY)
    return counts, (x, y, t, p, a, b, inst_idx, key)


def _assign_slots(pack, tiles_per_key):
    """Sort instances by (key, x) and assign (partition, tile-col) slots."""
    x, y, t, p, a, b, inst_idx, key = pack
    col0 = np.zeros(NKEY + 1, np.int64)
    col0[1:] = np.cumsum(tiles_per_key)
    order = np.lexsort((x[inst_idx], key))
    skey = key[order]
    sidx = inst_idx[order]
    group_start = np.searchsorted(skey, np.arange(NKEY))
    rank = np.arange(len(skey)) - group_start[skey]
    slot = col0[skey] * P + rank
    part = (slot % P).astype(np.int64)
    col = (slot // P).astype(np.int64)
    return part, col, sidx, (x, y, t, p, a, b)


def _pack_core(slots, base, T_tot, key_of_col):
    part, col, sidx, (x, y, t, p, a, b) = slots
    xv = x[sidx]
    yv = y[sidx]
    tv = t[sidx]
    pv = p[sidx]

    YX = np.zeros((P, 2 * T_tot), np.float32)
    # pads: y = stripe base row (safe: g=0), xloc = 0
    YX[:, 0:T_tot] = ((key_of_col % NQ8) * QS)[None, :].astype(np.float32)
    YX[part, col] = yv
    YX[part, T_tot + col] = xv - base[col].astype(np.float32)
    TP = np.zeros((P, 2 * T_tot + 2), np.float32)
    TP[part, col] = tv
    TP[part, T_tot + col] = pv
    TP[:, 2 * T_tot] = a
    TP[:, 2 * T_tot + 1] = b
    return {"ev_yx": YX, "ev_tp": TP}


def _build_program(tiles_per_key, base, width, T_tot, WXM):
    nc = bacc.Bacc("TRN2", debug=False)
    yx_d = nc.dram_tensor("ev_yx", [P, 2 * T_tot], F32, kind="ExternalInput")
    tp_d = nc.dram_tensor("ev_tp", [P, 2 * T_tot + 2], F32,
                          kind="ExternalInput")
    out_d = nc.dram_tensor("out", [BINS, H, W], F32, kind="ExternalOutput")

    col0 = np.zeros(NKEY + 1, np.int64)
    col0[1:] = np.cumsum(tiles_per_key)
    seg_c0 = [int(col0[s * NQ8]) for s in range(NSEG)]
    seg_c1 = [int(col0[(s + 1) * NQ8]) for s in range(NSEG)]

    Alu = mybir.AluOpType
    Act = mybir.ActivationFunctionType

    PSW = 1280            # psum cols: [0,640) bin s | [640,1280) bin s+1
    BANKS = (0, 512, 1024, 1280)

    with tile.TileContext(nc) as tc:
        with (
            tc.tile_pool(name="persist", bufs=1) as persist,
            tc.tile_pool(name="grid", bufs=1) as gridp,
            tc.tile_pool(name="psum", bufs=2, space="PSUM") as psump,
        ):
            yxt = persist.tile([P, 2 * T_tot], F32, tag="yxt")
            yt = yxt[:, 0:T_tot]
            xlt = yxt[:, T_tot:2 * T_tot]
            nc.sync.dma_start(out=yxt[:], in_=yx_d[:])

            # iota row 0..639 (f32), shared by y-windows and x-windows
            ioi = persist.tile([P, W], mybir.dt.int32, tag="ioi")
            nc.gpsimd.iota(ioi[:], pattern=[[1, W]], base=0,
                           channel_multiplier=0)
            iof = persist.tile([P, W], F32, tag="iof")
            nc.vector.tensor_copy(iof[:], ioi[:])

            # prologue: frac = a*t + b - s ; g1 = p*frac ; g0 = p - g1
            g0p = persist.tile([P, T_tot], F32, tag="g0p")
            g1p = persist.tile([P, T_tot], F32, tag="g1p")
            with tc.tile_pool(name="prolog", bufs=1) as prolog:
                tpt = prolog.tile([P, 2 * T_tot + 2], F32, tag="tpt")
                tt = tpt[:, 0:T_tot]
                pt = tpt[:, T_tot:2 * T_tot]
                ab = tpt[:, 2 * T_tot:2 * T_tot + 2]
                nc.sync.dma_start(out=tpt[:], in_=tp_d[:])
                tc.strict_bb_all_engine_barrier()
                frac = prolog.tile([P, T_tot], F32, tag="frac")
                nc.vector.tensor_scalar(frac[:], tt, ab[:, 0:1], ab[:, 1:2],
                                        op0=Alu.mult, op1=Alu.add)
                for s in range(NSEG):
                    c0, c1 = seg_c0[s], seg_c1[s]
                    if c1 > c0 and s > 0:
                        nc.vector.tensor_scalar(frac[:, c0:c1], frac[:, c0:c1],
                                                float(s), None,
                                                op0=Alu.subtract)
                nc.vector.tensor_tensor(g1p[:], pt, frac[:], op=Alu.mult)
                nc.vector.tensor_tensor(g0p[:], pt, g1p[:], op=Alu.subtract)

            # output grid: every (plane, q8) region is first written by a
            # copy-drain, so no zero-init is needed
            V = gridp.tile([P, BINS * NQ4 * W], F32, tag="V")
            z16 = persist.tile([P, 512], F16, tag="z16")
            nc.vector.memset(z16[:], 0.0)

            tc.strict_bb_all_engine_barrier()

            repeat = int(os.environ.get("EVS_REPEAT", "1"))
            dx_rr = 0
            with (
                tc.tile_pool(name="dyp", bufs=3) as dyp,
                tc.tile_pool(name="adyp", bufs=3) as adyp,
                tc.tile_pool(name="nhyp", bufs=3) as nhyp,
                tc.tile_pool(name="dxp", bufs=2) as dxp,
                tc.tile_pool(name="adxp", bufs=2) as adxp,
                tc.tile_pool(name="mxcp", bufs=2) as mxcp,
                tc.tile_pool(name="rp", bufs=8) as rp,
            ):
             for _rep in range(repeat):
              for s in range(NSEG):
                for q8 in range(NQ8):
                    k = s * NQ8 + q8
                    ntile = int(tiles_per_key[k])
                    cbase = int(col0[k])
                    qlo = q8 * QS

                    ps = psump.tile([P, PSW], F32, tag="ps")
                    pr0 = (q8 & 1) * QS
                    q4 = q8 >> 1

                    # which bank does each mm piece land in; track last per bank
                    bank_mms = {0: [], 1: [], 2: []}
                    for t_i in range(ntile):
                        c = cbase + t_i
                        w = int(width[c])
                        bs = int(base[c])
                        for half in range(2):
                            lo = half * W + bs
                            hi = lo + w
                            pieces = []
                            cur = lo
                            for bb in BANKS[1:]:
                                if cur < bb:
                                    pe = min(hi, bb)
                                    pieces.append((cur, pe))
                                    cur = pe
                                    if cur >= hi:
                                        break
                            for (p0, p1) in pieces:
                                bk = p0 // 512
                                bank_mms[bk].append((t_i, half, p0, p1))
                    last_set = {tuple(lst[-1]) for lst in bank_mms.values()
                                if lst}

                    # zero-fill each bank (start=True); stop only if no mms
                    for bk in range(3):
                        b0, b1 = BANKS[bk], BANKS[bk + 1]
                        nc.tensor.matmul(ps[pr0:pr0 + QS, b0:b1],
                                         lhsT=z16[:, 0:QS],
                                         rhs=z16[:, 0:b1 - b0],
                                         start=True,
                                         stop=(len(bank_mms[bk]) == 0))

                    g0 = 0
                    while g0 < ntile:
                        gn = 1
                        wg = int(width[cbase + g0])
                        while (g0 + gn < ntile and gn < GROUP):
                            w2 = max(wg, int(width[cbase + g0 + gn]))
                            if (gn + 1) * w2 > XCAP:
                                break
                            wg = w2
                            gn += 1
                        gstart = g0
                        c0 = cbase + gstart
                        g0 += gn

                        # y side: dy (Pool), ady (ACT), nhy16 (DVE)
                        dyS = dyp.tile([P, GROUP * QS], F32, tag="dyS")
                        io_y = iof[:, qlo:qlo + QS].unsqueeze(1) \
                            .broadcast_to([P, gn, QS])
                        y_b = yt[:, c0:c0 + gn].to_broadcast([P, gn, QS])
                        dy3 = dyS[:, 0:gn * QS].rearrange(
                            "p (g w) -> p g w", g=gn)
                        nc.gpsimd.tensor_tensor(dy3, io_y, y_b,
                                                op=Alu.subtract)
                        adyS = adyp.tile([P, GROUP * QS], F32, tag="adyS")
                        nc.scalar.activation(adyS[:, 0:gn * QS],
                                             dyS[:, 0:gn * QS], Act.Abs)
                        nhyS = nhyp.tile([P, GROUP * QS], F16, tag="nhyS")
                        nc.vector.tensor_scalar(nhyS[:, 0:gn * QS],
                                                adyS[:, 0:gn * QS], 1.0, 1.0,
                                                op0=Alu.min, op1=Alu.subtract)

                        # x side: dx (Pool/DVE round-robin), adx (ACT),
                        # mxc16 (DVE)
                        dxS = dxp.tile([P, XCAP], F32, tag="dxS")
                        io_x = iof[:, 0:wg].unsqueeze(1) \
                            .broadcast_to([P, gn, wg])
                        xl_b = xlt[:, c0:c0 + gn].to_broadcast([P, gn, wg])
                        dx3 = dxS[:, 0:gn * wg].rearrange(
                            "p (g w) -> p g w", g=gn)
                        if dx_rr < DX_POOL_NUM:
                            nc.gpsimd.tensor_tensor(dx3, io_x, xl_b,
                                                    op=Alu.subtract)
                        else:
                            nc.vector.tensor_tensor(dx3, io_x, xl_b,
                                                    op=Alu.subtract)
                        dx_rr = (dx_rr + 1) % DX_POOL_DEN
                        adxS = adxp.tile([P, XCAP], F32, tag="adxS")
                        nc.scalar.activation(adxS[:, 0:gn * wg],
                                             dxS[:, 0:gn * wg], Act.Abs)
                        mxcS = mxcp.tile([P, XCAP], F16, tag="mxcS")
                        nc.vector.tensor_scalar(mxcS[:, 0:gn * wg],
                                                adxS[:, 0:gn * wg], 1.0, None,
                                                op0=Alu.min)

                        for j in range(gn):
                            t_i = gstart + j
                            c = cbase + t_i
                            w = int(width[c])
                            bs = int(base[c])
                            mx_j = mxcS[:, j * wg:j * wg + w]
                            for half, gcol in ((0, g0p), (1, g1p)):
                                rr_t = rp.tile([P, 640], F16, tag="rr")
                                rrw = rr_t[:, 0:w]
                                nc.vector.tensor_scalar(
                                    rrw, mx_j, gcol[:, c:c + 1],
                                    gcol[:, c:c + 1],
                                    op0=Alu.mult, op1=Alu.subtract)
                                lo = half * W + bs
                                hi = lo + w
                                cur = lo
                                for bb in BANKS[1:]:
                                    if cur < bb:
                                        pe = min(hi, bb)
                                        is_last = (t_i, half, cur, pe) \
                                            in last_set
                                        nc.tensor.matmul(
                                            ps[pr0:pr0 + QS, cur:pe],
                                            lhsT=nhyS[:, j * QS:(j + 1) * QS],
                                            rhs=rr_t[:, cur - lo:pe - lo],
                                            start=False, stop=is_last)
                                        cur = pe
                                        if cur >= hi:
                                            break

                    # drain psum halves into V: first writer of a region
                    # copies (ACT), second adds (DVE)
                    for half, plane in ((0, s), (1, s + 1)):
                        vbase = (plane * NQ4 + q4) * W
                        vv = V[pr0:pr0 + QS, vbase:vbase + W]
                        pv = ps[pr0:pr0 + QS, half * W:(half + 1) * W]
                        if half == 1 or s == 0:
                            nc.scalar.copy(vv, pv)
                        else:
                            nc.vector.tensor_tensor(vv, vv, pv, op=Alu.add)

                # plane s final after its half-0 drains: stream it out now
                if _rep == repeat - 1:
                    planes = [s] if s < NSEG - 1 else [s, s + 1]
                    for bin_i in planes:
                        for q4 in range(NQ4):
                            rows = min(P, H - q4 * P)
                            vb = (bin_i * NQ4 + q4) * W
                            nc.sync.dma_start(
                                out=out_d[bin_i, q4 * P:q4 * P + rows, :],
                                in_=V[0:rows, vb:vb + W])
    nc.finalize()
    return nc


def kernel(events, lengths):
    events = np.ascontiguousarray(events, dtype=np.float32)
    lengths = np.asarray(lengths)
    B = int(lengths.shape[0])
    offs = np.zeros(B + 1, np.int64)
    offs[1:] = np.cumsum(lengths)

    packs = []
    counts = np.zeros((B, NKEY), np.int64)
    for bi in range(B):
        c, pk = _host_prep(events[offs[bi]:offs[bi + 1]])
        counts[bi] = c
        packs.append(pk)

    tiles_per_key = np.maximum(1, -(-counts.max(axis=0) // P)).astype(np.int64)
    T_tot = int(tiles_per_key.sum())
    col0 = np.zeros(NKEY + 1, np.int64)
    col0[1:] = np.cumsum(tiles_per_key)
    key_of_col = np.repeat(np.arange(NKEY), tiles_per_key)

    # per-core slot assignment, then cross-core per-tile x-window
    slots = [_assign_slots(pk, tiles_per_key) for pk in packs]
    minx = np.full(T_tot, W, np.int64)
    maxe = np.zeros(T_tot, np.int64)
    for part, col, sidx, (x, y, t, p, a, b) in slots:
        fx = np.floor(x[sidx]).astype(np.int64)
        np.minimum.at(minx, col, fx)
        np.maximum.at(maxe, col, fx + 2)
    base = np.minimum(minx, W - 4)
    base = np.maximum(base, 0)
    end = np.maximum(maxe, base + 4)
    end = np.minimum(end, W)
    width = end - base
    width = np.minimum((width + 1) // 2 * 2, W - base)

    WXM = int(width.max())
    key = (tuple(tiles_per_key.tolist()), tuple(base.tolist()),
           tuple(width.tolist()), os.environ.get("EVS_REPEAT", "1"))
    if key not in _prog_cache:
        _prog_cache[key] = _build_program(tiles_per_key, base, width, T_tot,
                                          WXM)
    nc = _prog_cache[key]

    in_maps = [_pack_core(sl, base, T_tot, key_of_col) for sl in slots]
    trace = bool(int(os.environ.get("EVS_TRACE", "0")))
    res = run_bass_kernel_spmd(nc, in_maps, core_ids=list(range(B)),
                               trace=trace)
    global last_results
    last_results = res
    out = np.stack([r["out"] for r in res.results], axis=0)
    return out.astype(np.float32)


last_results = None


if __name__ == "__main__":
    rng = np.random.default_rng(0)
    B0, NP0 = 8, 2000
    N0 = B0 * NP0
    x = rng.uniform(0, W - 1, N0).astype(np.float32)
    y = rng.uniform(0, H - 1, N0).astype(np.float32)
    t = np.sort(rng.uniform(0, 1, (B0, NP0)).astype(np.float32), axis=1).ravel()
    p = (2.0 * rng.integers(0, 2, N0) - 1).astype(np.float32)
    b = np.repeat(np.arange(B0), NP0).astype(np.float32)
    ev = np.stack([x, y, t, p, b], axis=1)
    ln = np.full(B0, NP0, np.int32)
    out = kernel(ev, ln)
    ref = np.zeros((B0, BINS, H, W), np.float64)
    for bi in range(B0):
        sl = slice(bi * NP0, (bi + 1) * NP0)
        xx, yy, tt2, pp = x[sl], y[sl], t[sl], p[sl]
        t0, tN = tt2[0], tt2[-1]
        ts = (BINS - 1) * np.clip((tt2 - t0) / (tN - t0), 0, 1)
        import itertools
        for xr_f, yr_f, br_f in itertools.product([np.floor, np.ceil], repeat=3):
            xr, yr, br = xr_f(xx), yr_f(yy), br_f(ts)
            valid = (((xr != xx) | (xr_f is np.floor))
                     & ((yr != yy) | (yr_f is np.floor))
                     & ((br != ts) | (br_f is np.floor))
                     & (xr < W) & (yr < H) & (br < BINS))
            kb = lambda a_: np.maximum(0, 1 - np.abs(a_))
            val = np.where(valid, pp * kb(xr - xx) * kb(yr - yy) * kb(br - ts), 0)
            np.add.at(ref[bi].ravel(),
                      np.where(valid, (xr + yr * W + br * H * W).astype(np.int64), 0),
                      val)
    err = np.abs(out - ref).max() / max(1e-9, np.abs(ref).max())
    print("smoke rel err:", err)


# revision 7
# speedup vs baseline: 1.5818x; 1.0740x over previous
"""EventVolumeSurface trilinear voxel-grid kernel for Trainium2 (Bass/Tile).

Strategy (data-parallel over batch, 1 batch -> 1 NeuronCore):
  Host: shard events by batch id, bucket by (time-segment s in [0,9),
  y-stripe q in [0,8) of 64 rows; events straddling a y-stripe boundary are
  duplicated - the trilinear hat masks the out-of-stripe tap), then SORT each
  bucket's events by x.  Cut into 128-event tiles; each tile gets a
  compile-time x-window [base_t, base_t+w_t) covering all 8 cores' taps
  (w_t ~ 16-40 cols).  Pack slot-major [128, T] arrays of y, x-base-relative,
  t, polarity.

  Device, per tile of 128 events (ops batched over groups of <=8 tiles):
    dy   = iota_y - y            (Pool,  [128, G*64])
    ady  = |dy|                  (ACT Abs)
    nhy  = min(ady,1) - 1        (DVE fused tensor_scalar, fp16) = -hat_y
    dx   = iota_w - xloc         (Pool/DVE)
    adx  = |dx|                  (ACT Abs)
    mxc  = min(adx,1)            (DVE, fp16)
    r0   = g0*mxc - g0           (DVE fused 2-scalar TS, fp16) = -g0*hat_x
    r1   = g1*mxc - g1           where g0 = p*(1-frac), g1 = p*frac
    psum[64, 1280] += nhy^T @ [r0 | r1]  (PE, N=w_t per bin half)
  so psum accumulates +g*hat_y*hat_x for the two adjacent bin planes.  PSUM
  is drained per (s,q) into an SBUF-resident [10,480,640] grid (pre-zeroed)
  which is DMA'd out plane-by-plane as planes finalize.
"""

import os
import sys

import numpy as np

sys.path.insert(0, "/opt/trn_rl_repo")

import concourse.bass as bass
import concourse.bacc as bacc
import concourse.mybir as mybir
import concourse.tile as tile
from concourse.bass_utils import run_bass_kernel_spmd

H, W, BINS = 480, 640, 10
NSEG = BINS - 1          # 9 time segments (events with t*=9 fold into seg 8)
P = 128
QS = 64                  # y-stripe height
NQ8 = 8                  # ceil(480/64) = 7.5 -> 8 stripes (last half-used)
NKEY = NSEG * NQ8        # 72 buckets
NQ4 = 4                  # V-grid column blocks of 128 rows
N_CORES = 8
GROUP = 8                # tiles per batched op
XCAP = 1280              # max f32 columns per batched x-op slab (SBUF budget)

F32 = mybir.dt.float32
F16 = mybir.dt.float16

# fraction of x-diff (dx) batched ops issued on Pool engine (rest on DVE)
DX_POOL_NUM = int(os.environ.get("EVS_DX_POOL_NUM", "9"))
DX_POOL_DEN = 10

_prog_cache: dict = {}


def _host_prep(ev):
    """Bucket one batch's events by (s, q64); returns counts and raw data."""
    if ev.shape[0] == 0:
        ev = np.array([[0.0, 0.0, 0.25, 0.0, 0.0],
                       [0.0, 0.0, 0.75, 0.0, 0.0]], np.float32)
    x = ev[:, 0].astype(np.float32)
    y = ev[:, 1].astype(np.float32)
    t = ev[:, 2].astype(np.float32)
    p = ev[:, 3].astype(np.float32)
    t0 = t[0]
    tN = t[-1]
    denom = np.float32(tN - t0)
    if denom > 0:
        a = np.float32(np.float32(NSEG) / denom)
    else:
        a = np.float32(0.0)
    b = np.float32(-t0 * a)
    tp = (t * a + b).astype(np.float32)
    s = np.clip(np.floor(tp).astype(np.int32), 0, NSEG - 1)

    iy = np.floor(y).astype(np.int32)
    qf = iy >> 6
    qc = (iy + 1) >> 6
    n = len(x)
    idx0 = np.arange(n, dtype=np.int64)
    ys = qf != qc
    inst_idx = np.concatenate([idx0, idx0[ys]])
    inst_q = np.concatenate([qf, qc[ys]])
    key = s[inst_idx] * NQ8 + inst_q
    counts = np.bincount(key, minlength=NKEY)
    return counts, (x, y, t, p, a, b, inst_idx, key)


def _assign_slots(pack, tiles_per_key):
    """Sort instances by (key, x) and assign (partition, tile-col) slots."""
    x, y, t, p, a, b, inst_idx, key = pack
    col0 = np.zeros(NKEY + 1, np.int64)
    col0[1:] = np.cumsum(tiles_per_key)
    order = np.lexsort((x[inst_idx], key))
    skey = key[order]
    sidx = inst_idx[order]
    group_start = np.searchsorted(skey, np.arange(NKEY + 1))
    nk = np.diff(group_start)                      # this core's bucket counts
    rank = np.arange(len(skey)) - group_start[skey]
    # proportional-rank cut: tile j gets ranks [ceil(j*n/T), ceil((j+1)*n/T))
    # so each core's tiles cover aligned x-quantiles (narrower shared window)
    tk = tiles_per_key[skey]
    nks = np.maximum(nk[skey], 1)
    tile_in_key = (rank * tk) // nks
    # position within tile
    j0 = -(-(tile_in_key * nks) // tk)             # ceil(j*n/T)
    part = (rank - j0).astype(np.int64)
    col = (col0[skey] + tile_in_key).astype(np.int64)
    assert part.max(initial=0) < P
    return part, col, sidx, (x, y, t, p, a, b)


def _pack_core(slots, base, T_tot, key_of_col):
    part, col, sidx, (x, y, t, p, a, b) = slots
    xv = x[sidx]
    yv = y[sidx]
    tv = t[sidx]
    pv = p[sidx]

    YX = np.zeros((P, 2 * T_tot), np.float32)
    # pads: y = stripe base row (safe: g=0), xloc = 0
    YX[:, 0:T_tot] = ((key_of_col % NQ8) * QS)[None, :].astype(np.float32)
    YX[part, col] = yv
    YX[part, T_tot + col] = xv - base[col].astype(np.float32)
    TP = np.zeros((P, 2 * T_tot + 2), np.float32)
    TP[part, col] = tv
    TP[part, T_tot + col] = pv
    TP[:, 2 * T_tot] = a
    TP[:, 2 * T_tot + 1] = b
    return {"ev_yx": YX, "ev_tp": TP}


def _windows(slots, T_tot):
    """Shared per-tile x-window [base, base+width) covering all cores."""
    minx = np.full(T_tot, W, np.int64)
    maxe = np.zeros(T_tot, np.int64)
    for part, col, sidx, (x, y, t, p, a, b) in slots:
        fx = np.floor(x[sidx]).astype(np.int64)
        np.minimum.at(minx, col, fx)
        np.maximum.at(maxe, col, fx + 2)
    base = np.maximum(np.minimum(minx, W - 4), 0)
    end = np.minimum(np.maximum(maxe, base + 4), W)
    width = np.minimum((end - base + 1) // 2 * 2, W - base)
    return base, width


def _build_program(tiles_per_key, base, width, T_tot, WXM):
    nc = bacc.Bacc("TRN2", debug=False)
    yx_d = nc.dram_tensor("ev_yx", [P, 2 * T_tot], F32, kind="ExternalInput")
    tp_d = nc.dram_tensor("ev_tp", [P, 2 * T_tot + 2], F32,
                          kind="ExternalInput")
    out_d = nc.dram_tensor("out", [BINS, H, W], F32, kind="ExternalOutput")

    col0 = np.zeros(NKEY + 1, np.int64)
    col0[1:] = np.cumsum(tiles_per_key)
    seg_c0 = [int(col0[s * NQ8]) for s in range(NSEG)]
    seg_c1 = [int(col0[(s + 1) * NQ8]) for s in range(NSEG)]

    Alu = mybir.AluOpType
    Act = mybir.ActivationFunctionType

    with tile.TileContext(nc) as tc:
        with (
            tc.tile_pool(name="persist", bufs=1) as persist,
            tc.tile_pool(name="psum", bufs=3, space="PSUM") as psump,
        ):
            yxt = persist.tile([P, 2 * T_tot], F32, tag="yxt")
            yt = yxt[:, 0:T_tot]
            xlt = yxt[:, T_tot:2 * T_tot]
            nc.sync.dma_start(out=yxt[:], in_=yx_d[:])

            # iota row 0..639 (f32), shared by y-windows and x-windows
            ioi = persist.tile([P, W], mybir.dt.int32, tag="ioi")
            nc.gpsimd.iota(ioi[:], pattern=[[1, W]], base=0,
                           channel_multiplier=0)
            iof = persist.tile([P, W], F32, tag="iof")
            nc.vector.tensor_copy(iof[:], ioi[:])

            # prologue: frac = a*t + b - s ; g1 = p*frac ; g0 = p - g1
            g0p = persist.tile([P, T_tot], F32, tag="g0p")
            g1p = persist.tile([P, T_tot], F32, tag="g1p")
            with tc.tile_pool(name="prolog", bufs=1) as prolog:
                tpt = prolog.tile([P, 2 * T_tot + 2], F32, tag="tpt")
                tt = tpt[:, 0:T_tot]
                pt = tpt[:, T_tot:2 * T_tot]
                ab = tpt[:, 2 * T_tot:2 * T_tot + 2]
                nc.sync.dma_start(out=tpt[:], in_=tp_d[:])
                tc.strict_bb_all_engine_barrier()
                frac = prolog.tile([P, T_tot], F32, tag="frac")
                nc.vector.tensor_scalar(frac[:], tt, ab[:, 0:1], ab[:, 1:2],
                                        op0=Alu.mult, op1=Alu.add)
                for s in range(NSEG):
                    c0, c1 = seg_c0[s], seg_c1[s]
                    if c1 > c0 and s > 0:
                        nc.vector.tensor_scalar(frac[:, c0:c1], frac[:, c0:c1],
                                                float(s), None,
                                                op0=Alu.subtract)
                nc.vector.tensor_tensor(g1p[:], pt, frac[:], op=Alu.mult)
                nc.vector.tensor_tensor(g0p[:], pt, g1p[:], op=Alu.subtract)

            # output grid: every (plane, q8) region is first written by a
            # copy-drain, so no zero-init is needed
            z16 = persist.tile([P, 512], F16, tag="z16")
            nc.vector.memset(z16[:], 0.0)

            tc.strict_bb_all_engine_barrier()

            repeat = int(os.environ.get("EVS_REPEAT", "1"))
            dx_rr = 0
            XBANKS = (0, 512, W)
            with (
                tc.tile_pool(name="dyp", bufs=3) as dyp,
                tc.tile_pool(name="adyp", bufs=3) as adyp,
                tc.tile_pool(name="nhyp", bufs=3) as nhyp,
                tc.tile_pool(name="dxp", bufs=2) as dxp,
                tc.tile_pool(name="adxp", bufs=2) as adxp,
                tc.tile_pool(name="mxcp", bufs=2) as mxcp,
                tc.tile_pool(name="rp", bufs=8) as rp,
                tc.tile_pool(name="stgp", bufs=2) as stgp,
            ):
             for _rep in range(repeat):
              for q8 in range(NQ8):
                pr0 = (q8 & 1) * QS
                qlo = q8 * QS
                rows = min(QS, H - q8 * QS)

                # plan all mm pieces for this q8 stripe: piece key =
                # (s, half, tile, p0, p1); per output plane+bank find the
                # last writer (gets stop=True) and which banks are untouched
                def pieces_of(s, half):
                    k = s * NQ8 + q8
                    out = []
                    for t_i in range(int(tiles_per_key[k])):
                        c = int(col0[k]) + t_i
                        w = int(width[c])
                        bs = int(base[c])
                        cur = bs
                        while cur < bs + w:
                            nxt_b = min(bs + w,
                                        next(b for b in XBANKS[1:] if b > cur))
                            out.append((t_i, cur, nxt_b))
                            cur = nxt_b
                    return out

                last_set = set()
                empty_banks = {}   # plane -> set of untouched bank indices
                for plane in range(BINS):
                    empty_banks[plane] = set()
                    for bk in range(2):
                        b0, b1 = XBANKS[bk], XBANKS[bk + 1]
                        writers = []
                        if plane >= 1:
                            writers += [(plane - 1, 1, t, p0, p1)
                                        for (t, p0, p1) in pieces_of(plane - 1, 1)
                                        if b0 <= p0 < b1]
                        if plane <= NSEG - 1:
                            writers += [(plane, 0, t, p0, p1)
                                        for (t, p0, p1) in pieces_of(plane, 0)
                                        if b0 <= p0 < b1]
                        if writers:
                            last_set.add(writers[-1])
                        else:
                            empty_banks[plane].add(bk)

                def new_plane_tile(plane):
                    ps = psump.tile([P, W], F32, tag="pp")
                    for bk in range(2):
                        b0, b1 = XBANKS[bk], XBANKS[bk + 1]
                        nc.tensor.matmul(ps[pr0:pr0 + QS, b0:b1],
                                         lhsT=z16[:, 0:QS],
                                         rhs=z16[:, 0:b1 - b0],
                                         start=True,
                                         stop=(bk in empty_banks[plane]))
                    return ps

                ptile = {0: new_plane_tile(0)}
                for s in range(NSEG):
                    ptile[s + 1] = new_plane_tile(s + 1)
                    k = s * NQ8 + q8
                    ntile = int(tiles_per_key[k])
                    cbase = int(col0[k])

                    g0 = 0
                    while g0 < ntile:
                        gn = 1
                        wg = int(width[cbase + g0])
                        while (g0 + gn < ntile and gn < GROUP):
                            w2 = max(wg, int(width[cbase + g0 + gn]))
                            if (gn + 1) * w2 > XCAP:
                                break
                            wg = w2
                            gn += 1
                        gstart = g0
                        c0 = cbase + gstart
                        g0 += gn

                        # y side: dy (Pool), ady (ACT), nhy16 (DVE)
                        dyS = dyp.tile([P, GROUP * QS], F32, tag="dyS")
                        io_y = iof[:, qlo:qlo + QS].unsqueeze(1) \
                            .broadcast_to([P, gn, QS])
                        y_b = yt[:, c0:c0 + gn].to_broadcast([P, gn, QS])
                        dy3 = dyS[:, 0:gn * QS].rearrange(
                            "p (g w) -> p g w", g=gn)
                        nc.gpsimd.tensor_tensor(dy3, io_y, y_b,
                                                op=Alu.subtract)
                        adyS = adyp.tile([P, GROUP * QS], F32, tag="adyS")
                        nc.scalar.activation(adyS[:, 0:gn * QS],
                                             dyS[:, 0:gn * QS], Act.Abs)
                        nhyS = nhyp.tile([P, GROUP * QS], F16, tag="nhyS")
                        nc.vector.tensor_scalar(nhyS[:, 0:gn * QS],
                                                adyS[:, 0:gn * QS], 1.0, 1.0,
                                                op0=Alu.min, op1=Alu.subtract)

                        # x side: dx (Pool/DVE round-robin), adx (ACT),
                        # mxc16 (DVE)
                        dxS = dxp.tile([P, XCAP], F32, tag="dxS")
                        io_x = iof[:, 0:wg].unsqueeze(1) \
                            .broadcast_to([P, gn, wg])
                        xl_b = xlt[:, c0:c0 + gn].to_broadcast([P, gn, wg])
                        dx3 = dxS[:, 0:gn * wg].rearrange(
                            "p (g w) -> p g w", g=gn)
                        if dx_rr < DX_POOL_NUM:
                            nc.gpsimd.tensor_tensor(dx3, io_x, xl_b,
                                                    op=Alu.subtract)
                        else:
                            nc.vector.tensor_tensor(dx3, io_x, xl_b,
                                                    op=Alu.subtract)
                        dx_rr = (dx_rr + 1) % DX_POOL_DEN
                        adxS = adxp.tile([P, XCAP], F32, tag="adxS")
                        nc.scalar.activation(adxS[:, 0:gn * wg],
                                             dxS[:, 0:gn * wg], Act.Abs)
                        mxcS = mxcp.tile([P, XCAP], F16, tag="mxcS")
                        nc.vector.tensor_scalar(mxcS[:, 0:gn * wg],
                                                adxS[:, 0:gn * wg], 1.0, None,
                                                op0=Alu.min)

                        for j in range(gn):
                            t_i = gstart + j
                            c = cbase + t_i
                            w = int(width[c])
                            bs = int(base[c])
                            mx_j = mxcS[:, j * wg:j * wg + w]
                            for half, gcol in ((0, g0p), (1, g1p)):
                                ps = ptile[s + half]
                                rr_t = rp.tile([P, 640], F16, tag="rr")
                                rrw = rr_t[:, 0:w]
                                nc.vector.tensor_scalar(
                                    rrw, mx_j, gcol[:, c:c + 1],
                                    gcol[:, c:c + 1],
                                    op0=Alu.mult, op1=Alu.subtract)
                                cur = bs
                                while cur < bs + w:
                                    pe = min(bs + w,
                                             next(b for b in XBANKS[1:]
                                                  if b > cur))
                                    is_last = (s, half, t_i, cur, pe) \
                                        in last_set
                                    nc.tensor.matmul(
                                        ps[pr0:pr0 + QS, cur:pe],
                                        lhsT=nhyS[:, j * QS:(j + 1) * QS],
                                        rhs=rr_t[:, cur - bs:pe - bs],
                                        start=False, stop=is_last)
                                    cur = pe

                    # plane s complete: ACT-copy psum -> staging, DMA out
                    stg = stgp.tile([P, W], F32, tag="stg")
                    nc.scalar.copy(stg[pr0:pr0 + QS, :],
                                   ptile[s][pr0:pr0 + QS, :])
                    if _rep == repeat - 1 and rows > 0:
                        nc.sync.dma_start(
                            out=out_d[s, q8 * QS:q8 * QS + rows, :],
                            in_=stg[pr0:pr0 + rows, :])
                    del ptile[s]

                stg = stgp.tile([P, W], F32, tag="stg")
                nc.scalar.copy(stg[pr0:pr0 + QS, :],
                               ptile[NSEG][pr0:pr0 + QS, :])
                if _rep == repeat - 1 and rows > 0:
                    nc.sync.dma_start(
                        out=out_d[NSEG, q8 * QS:q8 * QS + rows, :],
                        in_=stg[pr0:pr0 + rows, :])
                del ptile[NSEG]
    nc.finalize()
    return nc


def kernel(events, lengths):
    events = np.ascontiguousarray(events, dtype=np.float32)
    lengths = np.asarray(lengths)
    B = int(lengths.shape[0])
    offs = np.zeros(B + 1, np.int64)
    offs[1:] = np.cumsum(lengths)

    packs = []
    counts = np.zeros((B, NKEY), np.int64)
    for bi in range(B):
        c, pk = _host_prep(events[offs[bi]:offs[bi + 1]])
        counts[bi] = c
        packs.append(pk)

    tiles_per_key = np.maximum(1, -(-counts.max(axis=0) // P)).astype(np.int64)
    T_tot = int(tiles_per_key.sum())
    col0 = np.zeros(NKEY + 1, np.int64)
    col0[1:] = np.cumsum(tiles_per_key)
    key_of_col = np.repeat(np.arange(NKEY), tiles_per_key)

    # per-core slot assignment; per-core window bases, shared widths
    slots = [_assign_slots(pk, tiles_per_key) for pk in packs]
    base, width = _windows(slots, T_tot)

    WXM = int(width.max())
    key = (tuple(tiles_per_key.tolist()), tuple(base.tolist()),
           tuple(width.tolist()), os.environ.get("EVS_REPEAT", "1"))
    if key not in _prog_cache:
        _prog_cache[key] = _build_program(tiles_per_key, base, width, T_tot,
                                          WXM)
    nc = _prog_cache[key]

    in_maps = [_pack_core(sl, base, T_tot, key_of_col) for sl in slots]
    trace = bool(int(os.environ.get("EVS_TRACE", "0")))
    res = run_bass_kernel_spmd(nc, in_maps, core_ids=list(range(B)),
                               trace=trace)
    global last_results
    last_results = res
    out = np.stack([r["out"] for r in res.results], axis=0)
    return out.astype(np.float32)


last_results = None


if __name__ == "__main__":
    rng = np.random.default_rng(0)
    B0, NP0 = 8, 2000
    N0 = B0 * NP0
    x = rng.uniform(0, W - 1, N0).astype(np.float32)
    y = rng.uniform(0, H - 1, N0).astype(np.float32)
    t = np.sort(rng.uniform(0, 1, (B0, NP0)).astype(np.float32), axis=1).ravel()
    p = (2.0 * rng.integers(0, 2, N0) - 1).astype(np.float32)
    b = np.repeat(np.arange(B0), NP0).astype(np.float32)
    ev = np.stack([x, y, t, p, b], axis=1)
    ln = np.full(B0, NP0, np.int32)
    out = kernel(ev, ln)
    ref = np.zeros((B0, BINS, H, W), np.float64)
    for bi in range(B0):
        sl = slice(bi * NP0, (bi + 1) * NP0)
        xx, yy, tt2, pp = x[sl], y[sl], t[sl], p[sl]
        t0, tN = tt2[0], tt2[-1]
        ts = (BINS - 1) * np.clip((tt2 - t0) / (tN - t0), 0, 1)
        import itertools
        for xr_f, yr_f, br_f in itertools.product([np.floor, np.ceil], repeat=3):
            xr, yr, br = xr_f(xx), yr_f(yy), br_f(ts)
            valid = (((xr != xx) | (xr_f is np.floor))
                     & ((yr != yy) | (yr_f is np.floor))
                     & ((br != ts) | (br_f is np.floor))
                     & (xr < W) & (yr < H) & (br < BINS))
            kb = lambda a_: np.maximum(0, 1 - np.abs(a_))
            val = np.where(valid, pp * kb(xr - xx) * kb(yr - yy) * kb(br - ts), 0)
            np.add.at(ref[bi].ravel(),
                      np.where(valid, (xr + yr * W + br * H * W).astype(np.int64), 0),
                      val)
    err = np.abs(out - ref).max() / max(1e-9, np.abs(ref).max())
    print("smoke rel err:", err)
